# revision 5
# baseline (speedup 1.0000x reference)
"""LiteMLA (linear attention) Trainium2 kernel — fp8 DoubleRow, v2.

Full-input contract: kernel(**inputs) takes the unsharded tensors from
setup_inputs() and returns the full (16, 256, 64, 64) float32 output.

Strategy
--------
Data-parallel over batch: 16 batch elements -> 8 NeuronCores x 2 each.
Heavy matmuls in fp8e4m3 DoubleRow (256-deep contraction per pass, rhs
streamed at 2 fp8/cycle). The projection matrix is folded into the V
weights on the host (v' = (Wp'.Wv).x), so the kv state IS the projected
state.

v2 changes vs v1 (trace-driven):
 - x is shipped twice: chunk-major xc[128,32,2,128] feeds the K phase's
   stationary operand (compact 256B LDWEIGHTS rows -> overlaps with the
   matmul stream; v1's stride-4096 x chunks serialized LDW+MM at
   ~431ns/MM vs ~218 achievable), and wide xw[128,2,4096] feeds the Q
   phase's moving operand (DoubleRow needs a 3D [K,2,n] ifmap).
 - q is stored chunk-major q8c[128,32,2,128] so the Z phase's stationary
   operand is compact as well.
 - Casts are balanced ACT/DVE per pair (the only two engines with a
   PSUM port); Z(b-1) is zipped into K(b)'s window; the final batch's Z
   rotates over all four PSUM tiles so the tail drains at 2-engine rate.

The device ships z[b, p, i, 0:257] bf16 (n = i*128+p); the host does
y = z[:, :256]/z[:, 256] + BN bias and transposes back to (B, C, H, W).
"""

import numpy as np
import ml_dtypes

import concourse.bass as bass
from concourse import bacc
import concourse.mybir as mybir
import concourse.tile as tile
from concourse.bass_utils import run_bass_kernel_spmd

B, C, H, W = 16, 256, 64, 64
N = H * W            # 4096
NCORES = 8
BL = B // NCORES     # batch elements per core
NT = N // 128        # 32 n-chunks
NPAIR = NT // 2      # 16 n-pair chunks
S = 1.0 / 64.0       # fp8 state scale (cancels in z/den)

BF16 = mybir.dt.bfloat16
F32 = mybir.dt.float32
FP8 = mybir.dt.float8e4
NPBF16 = ml_dtypes.bfloat16
NPFP8 = ml_dtypes.float8_e4m3

_CACHE = {}


def _build_program():
    nc = bacc.Bacc("TRN2", target_bir_lowering=False, debug=False)

    xc = nc.dram_tensor("xc8", [BL, 128, NT, 2, 128], FP8, kind="ExternalInput")
    xw = nc.dram_tensor("xw8", [BL, 128, 2, N], FP8, kind="ExternalInput")
    wq = nc.dram_tensor("wq8", [128, 2, C], FP8, kind="ExternalInput")
    wkv = nc.dram_tensor("wkv8", [128, 2, 2 * C], FP8, kind="ExternalInput")
    zs = nc.dram_tensor("z", [BL, 128, NT, C + 1], BF16, kind="ExternalOutput")

    Relu = mybir.ActivationFunctionType.Relu
    Copy = mybir.ActivationFunctionType.Copy
    DR = mybir.MatmulPerfMode.DoubleRow

    with tile.TileContext(nc) as tc:
        with (
            tc.tile_pool(name="const", bufs=1) as cp,
            tc.tile_pool(name="xcp", bufs=1) as xcp,
            tc.tile_pool(name="xwp", bufs=1) as xwp,
            tc.tile_pool(name="qp", bufs=2) as qp,
            tc.tile_pool(name="ktp", bufs=1) as ktp,
            tc.tile_pool(name="vtp", bufs=1) as vtp,
            tc.tile_pool(name="small", bufs=2) as sp,
            tc.tile_pool(name="hout", bufs=3) as hp,
            tc.tile_pool(name="ps_kq", bufs=2, space="PSUM") as ps_kq,
            tc.tile_pool(name="ps_z", bufs=2, space="PSUM") as ps_z,
        ):
            # ---------- input DMAs, weights first so MM #1 can start ----
            wq_sb = cp.tile([128, 2, C], FP8, tag="wq", name="wq")
            wkv_sb = cp.tile([128, 2, 2 * C], FP8, tag="wkv", name="wkv")
            xc_sb, xw_sb = {}, {}
            for bb in range(BL):
                xc_sb[bb] = xcp.tile([128, NT, 2, 128], FP8, tag=f"xc{bb}",
                                     name=f"xc_{bb}")
                xw_sb[bb] = xwp.tile([128, 2, N], FP8, tag=f"xw{bb}",
                                     name=f"xw_{bb}")
            nc.sync.dma_start(out=wkv_sb[:], in_=wkv[:])
            nc.sync.dma_start(out=xc_sb[0][:, 0:16], in_=xc[0, :, 0:16])
            nc.sync.dma_start(out=wq_sb[:], in_=wq[:])
            nc.sync.dma_start(out=xc_sb[0][:, 16:NT], in_=xc[0, :, 16:NT])
            nc.sync.dma_start(out=xw_sb[0][:], in_=xw[0])
            nc.sync.dma_start(out=xc_sb[1][:], in_=xc[1])
            nc.sync.dma_start(out=xw_sb[1][:], in_=xw[1])

            # kt/vt staging tiles, shared across batches; ones column of
            # vt written once, never touched again
            kt8 = [ktp.tile([128, 2, C], FP8, tag=f"kt{p}", name=f"kt_{p}")
                   for p in range(NPAIR)]
            vt8 = [vtp.tile([128, 2, C + 1], FP8, tag=f"vt{p}", name=f"vt_{p}")
                   for p in range(NPAIR)]
            for p in range(NPAIR):
                for j in range(2):
                    nc.gpsimd.memset(vt8[p][:, j, C:C + 1], 1.0)

            # ---------- Z-phase pair emitter --------------------------
            zstate = {}

            def emit_z_pair(zb, q8z, m8z, p, pool, tag):
                if p % 4 == 0:
                    zstate["sb"] = hp.tile([128, 8, C + 1], BF16, tag="z",
                                           name=f"z_{zb}_{p}")
                z_sb = zstate["sb"]
                jp = p % 4
                zps = pool.tile([128, 2, 512], F32, tag=tag,
                                name=f"zps_{zb}_{p}")
                for j in range(2):
                    nc.tensor.matmul(zps[:, j, 0:C + 1],
                                     lhsT=q8z[:, 2 * p + j, :, :],
                                     rhs=m8z[:],
                                     start=True, stop=True, perf_mode=DR)
                if p % 2 == 0:
                    nc.scalar.activation(z_sb[:, 2 * jp:2 * jp + 2, :],
                                         zps[:, :, 0:C + 1], Copy)
                else:
                    nc.vector.tensor_copy(z_sb[:, 2 * jp:2 * jp + 2, :],
                                          zps[:, :, 0:C + 1])
                if jp == 3:
                    g0 = p - 3
                    nc.sync.dma_start(out=zs[zb, :, 2 * g0:2 * g0 + 8, :],
                                      in_=z_sb[:])

            # ---------- Q-phase emitter (wide rhs, chunk-major out) ----
            def emit_q(b, q8c, qi):
                mc, iw2 = qi // 4, qi % 4
                qps = ps_kq.tile([128, 2, 512], F32, tag="kq",
                                 name=f"qps_{b}_{qi}")
                for j in range(2):
                    nc.tensor.matmul(
                        qps[:, j, :],
                        lhsT=wq_sb[:, :, mc * 128:(mc + 1) * 128],
                        rhs=xw_sb[b][:, :, (iw2 * 2 + j) * 512:(iw2 * 2 + j + 1) * 512],
                        start=True, stop=True, perf_mode=DR)
                # dst: chunks 8*iw2 + 4*j + h, c-half mc  (4D view of src)
                src = qps[:, :, :].rearrange("p j (h r) -> p j h r", r=128)
                dst = q8c[:, 8 * iw2:8 * iw2 + 8, mc, :].rearrange(
                    "p (j h) r -> p j h r", j=2)
                if qi % 2 == 0:
                    nc.scalar.activation(dst, src, Relu)
                else:
                    nc.vector.tensor_scalar_max(dst, src, 0.0)

            prev = None  # (q8c, m8) of the previous batch
            for b in range(BL):
                # ---------- window A: K(b) zipped with Z(b-1) ----------
                for p in range(NPAIR):
                    kvps = ps_kq.tile([128, 2, 512], F32, tag="kq",
                                      name=f"kvps_{b}_{p}")
                    for j in range(2):
                        nc.tensor.matmul(kvps[:, j, :],
                                         lhsT=xc_sb[b][:, 2 * p + j, :, :],
                                         rhs=wkv_sb[:], start=True, stop=True,
                                         perf_mode=DR)
                    if p % 2 == 0:
                        nc.scalar.activation(kt8[p][:, :, :],
                                             kvps[:, :, 0:C], Relu)
                        nc.vector.tensor_copy(vt8[p][:, :, 0:C],
                                              kvps[:, :, C:2 * C])
                    else:
                        nc.vector.tensor_scalar_max(kt8[p][:, :, :],
                                                    kvps[:, :, 0:C], 0.0)
                        nc.scalar.activation(vt8[p][:, :, 0:C],
                                             kvps[:, :, C:2 * C], Copy)
                    if prev is not None:
                        emit_z_pair(b - 1, prev[0], prev[1], p, ps_z, "ps_z")

                # ---------- window B: kv(b) zipped with Q(b) -----------
                q8c = qp.tile([128, NT, 2, 128], FP8, tag="q", name=f"q_{b}")
                kv_ps = ps_z.tile([128, 2, 512], F32, tag="ps_z",
                                  name=f"kv_{b}")
                for p in range(NPAIR):
                    for cc in range(2):
                        nc.tensor.matmul(kv_ps[:, cc, 0:C + 1],
                                         lhsT=kt8[p][:, :, cc * 128:(cc + 1) * 128],
                                         rhs=vt8[p][:],
                                         start=(p == 0), stop=(p == NPAIR - 1),
                                         perf_mode=DR)
                    if p % 2 == 1:
                        emit_q(b, q8c, p // 2)
                m8 = sp.tile([128, 2, C + 1], FP8, tag="m8", name=f"m8_{b}")
                nc.scalar.activation(m8[:], kv_ps[:, :, 0:C + 1], Copy, scale=S)
                prev = (q8c, m8)

            # ---------- final batch's Z phase over both PSUM pools -----
            for p in range(NPAIR):
                if p % 2 == 0:
                    emit_z_pair(BL - 1, prev[0], prev[1], p, ps_z, "ps_z")
                else:
                    emit_z_pair(BL - 1, prev[0], prev[1], p, ps_kq, "kq")
    nc.compile()
    return nc


def _prep_inputs(x, w_qkv, w_proj, bn_gamma, bn_beta, bn_mean, bn_var):
    x = np.asarray(x, dtype=np.float32)
    w_qkv = np.asarray(w_qkv, dtype=np.float32)
    w_proj = np.asarray(w_proj, dtype=np.float32)
    bn_gamma = np.asarray(bn_gamma, dtype=np.float32)
    bn_beta = np.asarray(bn_beta, dtype=np.float32)
    bn_mean = np.asarray(bn_mean, dtype=np.float32)
    bn_var = np.asarray(bn_var, dtype=np.float32)

    # torch-faithful interleave: out-channel 3*i+j -> (channel i, {q,k,v}[j])
    def w8(wm):  # (C_out, C_in) -> [128, 2, C_out] fp8: [p, j, o] = w[o, j*128+p]
        return np.ascontiguousarray(
            wm.T.reshape(2, 128, -1).transpose(1, 0, 2).astype(NPFP8))

    scale = bn_gamma / np.sqrt(bn_var + 1e-5)
    wq8 = w8(w_qkv[0::3])
    # combined [wk | Wp'.Wv] along the output dim (projection folded into V)
    wpv = (scale[:, None] * w_proj) @ w_qkv[2::3]
    wkv8 = w8(np.concatenate([w_qkv[1::3], wpv], axis=0))
    x8 = x.reshape(B, 2, 128, N).astype(NPFP8)
    # wide: [b, p, j, n] = x[b, j*128+p, n]
    xw8 = np.ascontiguousarray(x8.transpose(0, 2, 1, 3))
    # chunk-major: [b, p, i, j, r] = x[b, j*128+p, i*128+r]
    xc8 = np.ascontiguousarray(
        x8.reshape(B, 2, 128, NT, 128).transpose(0, 2, 3, 1, 4))

    bias = (bn_beta - bn_mean * scale).astype(np.float32)

    in_maps = []
    for core in range(NCORES):
        in_maps.append({
            "xc8": xc8[core * BL:(core + 1) * BL],
            "xw8": xw8[core * BL:(core + 1) * BL],
            "wq8": wq8, "wkv8": wkv8,
        })
    return in_maps, bias


def _postprocess(z_raw, bias):
    # z_raw: (B, 128, NT, C+1), n = i*128+p -> y (B, C, H, W) f32
    z = z_raw.transpose(0, 2, 1, 3).reshape(B, N, C + 1)
    y = z[:, :, :C] / z[:, :, C:C + 1] + bias[None, None, :]
    return np.ascontiguousarray(y.transpose(0, 2, 1)).reshape(B, C, H, W)


def _run(inputs, trace=False, **kw):
    if "nc" not in _CACHE:
        _CACHE["nc"] = _build_program()
    nc = _CACHE["nc"]
    in_maps, bias = _prep_inputs(**inputs)
    res = run_bass_kernel_spmd(nc, in_maps, list(range(NCORES)), trace=trace, **kw)
    z_raw = np.concatenate([res.results[i]["z"] for i in range(NCORES)], axis=0)
    return _postprocess(z_raw.astype(np.float32), bias), res


def kernel(**inputs):
    y, _ = _run(inputs)
    return y


# revision 6
# speedup vs baseline: 1.0726x; 1.0726x over previous
"""LiteMLA (linear attention) Trainium2 kernel — fp8 DoubleRow, v3.

Full-input contract: kernel(**inputs) takes the unsharded tensors from
setup_inputs() and returns the full (16, 256, 64, 64) float32 output.

Strategy
--------
Data-parallel over batch: 16 batch elements -> 8 NeuronCores x 2 each.
Heavy matmuls in fp8e4m3 DoubleRow (256-deep contraction per pass, rhs
streamed at 2 fp8/cycle). The projection matrix is folded into the V
weights on the host (v' = (Wp'.Wv).x), so the kv state IS the projected
state.

v3 = v1's single 4-deep PSUM rotation (2-deep per-pool ping-pong in v2
convoyed on sem latency) + the v2 trace-driven matmul fixes:
 - x ships twice: chunk-major xc[128,32,2,128] feeds the K phase's
   stationary operand (compact 256B LDWEIGHTS rows overlap the matmul
   stream; stride-4096 chunks serialized LDW+MM at ~431ns/MM vs ~218),
   wide xw[128,2,4096] feeds the Q phase's moving operand (DoubleRow
   needs a 3D [K,2,n] ifmap).
 - q is stored chunk-major q8c[128,32,2,128] so the Z phase's
   stationary operand is compact (Z paces at ~116ns/MM vs ~334).
 - kt/vt/z/q casts alternate ACT/DVE per pair (the only two engines
   with a PSUM port).

The device ships z[b, p, i, 0:257] bf16 (n = i*128+p); the host does
y = z[:, :256]/z[:, 256] + BN bias and transposes back to (B, C, H, W).
"""

import numpy as np
import ml_dtypes

import concourse.bass as bass
from concourse import bacc
import concourse.mybir as mybir
import concourse.tile as tile
from concourse.bass_utils import run_bass_kernel_spmd

B, C, H, W = 16, 256, 64, 64
N = H * W            # 4096
NCORES = 8
BL = B // NCORES     # batch elements per core
NT = N // 128        # 32 n-chunks
NPAIR = NT // 2      # 16 n-pair chunks
S = 1.0 / 64.0       # fp8 state scale (cancels in z/den)

BF16 = mybir.dt.bfloat16
F32 = mybir.dt.float32
FP8 = mybir.dt.float8e4
NPBF16 = ml_dtypes.bfloat16
NPFP8 = ml_dtypes.float8_e4m3

_CACHE = {}


def _build_program():
    nc = bacc.Bacc("TRN2", target_bir_lowering=False, debug=False)

    xc = nc.dram_tensor("xc8", [BL, 128, NT, 2, 128], FP8, kind="ExternalInput")
    xw = nc.dram_tensor("xw8", [BL, 128, 2, N], FP8, kind="ExternalInput")
    wq = nc.dram_tensor("wq8", [128, 2, C], FP8, kind="ExternalInput")
    wkv = nc.dram_tensor("wkv8", [128, 2, 2 * C], FP8, kind="ExternalInput")
    zs = nc.dram_tensor("z", [BL, 128, NT, C + 1], BF16, kind="ExternalOutput")

    Relu = mybir.ActivationFunctionType.Relu
    Copy = mybir.ActivationFunctionType.Copy
    DR = mybir.MatmulPerfMode.DoubleRow

    with tile.TileContext(nc) as tc:
        with (
            tc.tile_pool(name="const", bufs=1) as cp,
            tc.tile_pool(name="xcp", bufs=1) as xcp,
            tc.tile_pool(name="xwp", bufs=1) as xwp,
            tc.tile_pool(name="qp", bufs=2) as qp,
            tc.tile_pool(name="ktp", bufs=1) as ktp,
            tc.tile_pool(name="vtp", bufs=1) as vtp,
            tc.tile_pool(name="small", bufs=2) as sp,
            tc.tile_pool(name="hout", bufs=3) as hp,
            tc.tile_pool(name="ps_big", bufs=4, space="PSUM") as ps_big,
        ):
            # ---------- input DMAs, weights first so MM #1 can start ----
            wq_sb = cp.tile([128, 2, C], FP8, tag="wq", name="wq")
            wkv_sb = cp.tile([128, 2, 2 * C], FP8, tag="wkv", name="wkv")
            xc_sb, xw_sb = {}, {}
            for bb in range(BL):
                xc_sb[bb] = xcp.tile([128, NT, 2, 128], FP8, tag=f"xc{bb}",
                                     name=f"xc_{bb}")
                xw_sb[bb] = xwp.tile([128, 2, N], FP8, tag=f"xw{bb}",
                                     name=f"xw_{bb}")
            nc.sync.dma_start(out=wkv_sb[:], in_=wkv[:])
            nc.sync.dma_start(out=xc_sb[0][:, 0:8], in_=xc[0, :, 0:8])
            nc.sync.dma_start(out=wq_sb[:], in_=wq[:])
            nc.sync.dma_start(out=xc_sb[0][:, 8:NT], in_=xc[0, :, 8:NT])
            nc.sync.dma_start(out=xc_sb[1][:], in_=xc[1])
            nc.sync.dma_start(out=xw_sb[0][:], in_=xw[0])
            nc.sync.dma_start(out=xw_sb[1][:], in_=xw[1])

            # kt/vt staging tiles, shared across batches; ones column of
            # vt written once, never touched again
            kt8 = [ktp.tile([128, 2, C], FP8, tag=f"kt{p}", name=f"kt_{p}")
                   for p in range(NPAIR)]
            vt8 = [vtp.tile([128, 2, C + 1], FP8, tag=f"vt{p}", name=f"vt_{p}")
                   for p in range(NPAIR)]
            for p in range(NPAIR):
                for j in range(2):
                    nc.gpsimd.memset(vt8[p][:, j, C:C + 1], 1.0)

            # ---------- Z-phase pair emitter --------------------------
            zstate = {}

            def emit_z_pair(zb, q8z, m8z, p, glen=8):
                if p % (glen // 2) == 0:
                    zstate["sb"] = hp.tile([128, glen, C + 1], BF16, tag="z",
                                           name=f"z_{zb}_{p}")
                z_sb = zstate["sb"]
                jp = p % (glen // 2)
                zps = ps_big.tile([128, 2, 512], F32, tag="big",
                                  name=f"zps_{zb}_{p}")
                for j in range(2):
                    nc.tensor.matmul(zps[:, j, 0:C + 1],
                                     lhsT=q8z[:, 2 * p + j, :, :],
                                     rhs=m8z[:],
                                     start=True, stop=True, perf_mode=DR)
                if p % 2 == 0:
                    nc.scalar.activation(z_sb[:, 2 * jp:2 * jp + 2, :],
                                         zps[:, :, 0:C + 1], Copy)
                else:
                    nc.vector.tensor_copy(z_sb[:, 2 * jp:2 * jp + 2, :],
                                          zps[:, :, 0:C + 1])
                if jp == glen // 2 - 1:
                    g0 = p - jp
                    nc.sync.dma_start(out=zs[zb, :, 2 * g0:2 * g0 + glen, :],
                                      in_=z_sb[:])

            # ---------- Q-phase emitter (wide rhs, chunk-major out) ----
            def emit_q(b, q8c, qi):
                mc, iw2 = qi // 4, qi % 4
                qps = ps_big.tile([128, 2, 512], F32, tag="big",
                                  name=f"qps_{b}_{qi}")
                for j in range(2):
                    nc.tensor.matmul(
                        qps[:, j, :],
                        lhsT=wq_sb[:, :, mc * 128:(mc + 1) * 128],
                        rhs=xw_sb[b][:, :, (iw2 * 2 + j) * 512:(iw2 * 2 + j + 1) * 512],
                        start=True, stop=True, perf_mode=DR)
                # dst: chunks 8*iw2 + 4*j + h, c-half mc  (4D view of src)
                src = qps[:, :, :].rearrange("p j (h r) -> p j h r", r=128)
                dst = q8c[:, 8 * iw2:8 * iw2 + 8, mc, :].rearrange(
                    "p (j h) r -> p j h r", j=2)
                if qi % 2 == 0:
                    nc.scalar.activation(dst, src, Relu)
                else:
                    nc.vector.tensor_scalar_max(dst, src, 0.0)

            prev = None  # (q8c, m8) of the previous batch
            for b in range(BL):
                # ---------- window A: K(b) zipped with Z(b-1) ----------
                for p in range(NPAIR):
                    kvps = ps_big.tile([128, 2, 512], F32, tag="big",
                                       name=f"kvps_{b}_{p}")
                    for j in range(2):
                        nc.tensor.matmul(kvps[:, j, :],
                                         lhsT=xc_sb[b][:, 2 * p + j, :, :],
                                         rhs=wkv_sb[:], start=True, stop=True,
                                         perf_mode=DR)
                    if p % 2 == 0:
                        nc.scalar.activation(kt8[p][:, :, :],
                                             kvps[:, :, 0:C], Relu)
                        nc.vector.tensor_copy(vt8[p][:, :, 0:C],
                                              kvps[:, :, C:2 * C])
                    else:
                        nc.vector.tensor_scalar_max(kt8[p][:, :, :],
                                                    kvps[:, :, 0:C], 0.0)
                        nc.scalar.activation(vt8[p][:, :, 0:C],
                                             kvps[:, :, C:2 * C], Copy)
                    if prev is not None:
                        emit_z_pair(b - 1, prev[0], prev[1], p)

                # ---------- window B: kv(b) zipped with Q(b) -----------
                q8c = qp.tile([128, NT, 2, 128], FP8, tag="q", name=f"q_{b}")
                kv_ps = ps_big.tile([128, 2, 512], F32, tag="big",
                                    name=f"kv_{b}")
                for p in range(NPAIR):
                    for cc in range(2):
                        nc.tensor.matmul(kv_ps[:, cc, 0:C + 1],
                                         lhsT=kt8[p][:, :, cc * 128:(cc + 1) * 128],
                                         rhs=vt8[p][:],
                                         start=(p == 0), stop=(p == NPAIR - 1),
                                         perf_mode=DR)
                    if p % 2 == 1:
                        emit_q(b, q8c, p // 2)
                m8 = sp.tile([128, 2, C + 1], FP8, tag="m8", name=f"m8_{b}")
                nc.scalar.activation(m8[:], kv_ps[:, :, 0:C + 1], Copy, scale=S)
                prev = (q8c, m8)

            # ---------- final batch's Z phase, tapered groups ----------
            for p in range(NPAIR):
                emit_z_pair(BL - 1, prev[0], prev[1], p, glen=4)
    nc.compile()
    return nc


def _prep_inputs(x, w_qkv, w_proj, bn_gamma, bn_beta, bn_mean, bn_var):
    x = np.asarray(x, dtype=np.float32)
    w_qkv = np.asarray(w_qkv, dtype=np.float32)
    w_proj = np.asarray(w_proj, dtype=np.float32)
    bn_gamma = np.asarray(bn_gamma, dtype=np.float32)
    bn_beta = np.asarray(bn_beta, dtype=np.float32)
    bn_mean = np.asarray(bn_mean, dtype=np.float32)
    bn_var = np.asarray(bn_var, dtype=np.float32)

    # torch-faithful interleave: out-channel 3*i+j -> (channel i, {q,k,v}[j])
    def w8(wm):  # (C_out, C_in) -> [128, 2, C_out] fp8: [p, j, o] = w[o, j*128+p]
        return np.ascontiguousarray(
            wm.T.reshape(2, 128, -1).transpose(1, 0, 2).astype(NPFP8))

    scale = bn_gamma / np.sqrt(bn_var + 1e-5)
    wq8 = w8(w_qkv[0::3])
    # combined [wk | Wp'.Wv] along the output dim (projection folded into V)
    wpv = (scale[:, None] * w_proj) @ w_qkv[2::3]
    wkv8 = w8(np.concatenate([w_qkv[1::3], wpv], axis=0))
    x8 = x.reshape(B, 2, 128, N).astype(NPFP8)
    # wide: [b, p, j, n] = x[b, j*128+p, n]
    xw8 = np.ascontiguousarray(x8.transpose(0, 2, 1, 3))
    # chunk-major: [b, p, i, j, r] = x[b, j*128+p, i*128+r]
    xc8 = np.ascontiguousarray(
        x8.reshape(B, 2, 128, NT, 128).transpose(0, 2, 3, 1, 4))

    bias = (bn_beta - bn_mean * scale).astype(np.float32)

    in_maps = []
    for core in range(NCORES):
        in_maps.append({
            "xc8": xc8[core * BL:(core + 1) * BL],
            "xw8": xw8[core * BL:(core + 1) * BL],
            "wq8": wq8, "wkv8": wkv8,
        })
    return in_maps, bias


def _postprocess(z_raw, bias):
    # z_raw: (B, 128, NT, C+1), n = i*128+p -> y (B, C, H, W) f32
    z = z_raw.transpose(0, 2, 1, 3).reshape(B, N, C + 1)
    y = z[:, :, :C] / z[:, :, C:C + 1] + bias[None, None, :]
    return np.ascontiguousarray(y.transpose(0, 2, 1)).reshape(B, C, H, W)


def _run(inputs, trace=False, **kw):
    if "nc" not in _CACHE:
        _CACHE["nc"] = _build_program()
    nc = _CACHE["nc"]
    in_maps, bias = _prep_inputs(**inputs)
    res = run_bass_kernel_spmd(nc, in_maps, list(range(NCORES)), trace=trace, **kw)
    z_raw = np.concatenate([res.results[i]["z"] for i in range(NCORES)], axis=0)
    return _postprocess(z_raw.astype(np.float32), bias), res


def kernel(**inputs):
    y, _ = _run(inputs)
    return y


# revision 10
# speedup vs baseline: 1.1553x; 1.0771x over previous
"""LiteMLA (linear attention) Trainium2 kernel — fp8 DoubleRow, v3.

Full-input contract: kernel(**inputs) takes the unsharded tensors from
setup_inputs() and returns the full (16, 256, 64, 64) float32 output.

Strategy
--------
Data-parallel over batch: 16 batch elements -> 8 NeuronCores x 2 each.
Heavy matmuls in fp8e4m3 DoubleRow (256-deep contraction per pass, rhs
streamed at 2 fp8/cycle). The projection matrix is folded into the V
weights on the host (v' = (Wp'.Wv).x), so the kv state IS the projected
state.

v3 = v1's single 4-deep PSUM rotation (2-deep per-pool ping-pong in v2
convoyed on sem latency) + the v2 trace-driven matmul fixes:
 - x ships twice: chunk-major xc[128,32,2,128] feeds the K phase's
   stationary operand (compact 256B LDWEIGHTS rows overlap the matmul
   stream; stride-4096 chunks serialized LDW+MM at ~431ns/MM vs ~218),
   wide xw[128,2,4096] feeds the Q phase's moving operand (DoubleRow
   needs a 3D [K,2,n] ifmap).
 - q is stored chunk-major q8c[128,32,2,128] so the Z phase's
   stationary operand is compact (Z paces at ~116ns/MM vs ~334).
 - kt/vt/z/q casts alternate ACT/DVE per pair (the only two engines
   with a PSUM port).

The device ships z[b, p, i, 0:257] bf16 (n = i*128+p); the host does
y = z[:, :256]/z[:, 256] + BN bias and transposes back to (B, C, H, W).
"""

import numpy as np
import ml_dtypes

import concourse.bass as bass
from concourse import bacc
import concourse.mybir as mybir
import concourse.tile as tile
from concourse.bass_utils import run_bass_kernel_spmd

B, C, H, W = 16, 256, 64, 64
N = H * W            # 4096
NCORES = 8
BL = B // NCORES     # batch elements per core
NT = N // 128        # 32 n-chunks
NPAIR = NT // 2      # 16 n-pair chunks
S = 1.0 / 64.0       # fp8 state scale (cancels in z/den)

BF16 = mybir.dt.bfloat16
F32 = mybir.dt.float32
FP8 = mybir.dt.float8e4
NPBF16 = ml_dtypes.bfloat16
NPFP8 = ml_dtypes.float8_e4m3

_CACHE = {}


def _build_program():
    nc = bacc.Bacc("TRN2", target_bir_lowering=False, debug=False)

    xc = nc.dram_tensor("xc8", [BL, 128, NT, 2, 128], FP8, kind="ExternalInput")
    xw = nc.dram_tensor("xw8", [BL, 128, 2, N], FP8, kind="ExternalInput")
    wq = nc.dram_tensor("wq8", [128, 2, C], FP8, kind="ExternalInput")
    wkv = nc.dram_tensor("wkv8", [128, 2, 2 * C], FP8, kind="ExternalInput")
    zs = nc.dram_tensor("z", [BL, 128, NT, C + 1], BF16, kind="ExternalOutput")

    Relu = mybir.ActivationFunctionType.Relu
    Copy = mybir.ActivationFunctionType.Copy
    DR = mybir.MatmulPerfMode.DoubleRow

    with tile.TileContext(nc) as tc:
        with (
            tc.tile_pool(name="const", bufs=1) as cp,
            tc.tile_pool(name="xcp", bufs=1) as xcp,
            tc.tile_pool(name="xwp", bufs=1) as xwp,
            tc.tile_pool(name="qp", bufs=2) as qp,
            tc.tile_pool(name="ktp", bufs=1) as ktp,
            tc.tile_pool(name="vtp", bufs=1) as vtp,
            tc.tile_pool(name="small", bufs=2) as sp,
            tc.tile_pool(name="hout", bufs=4) as hp,
            tc.tile_pool(name="ps_big", bufs=4, space="PSUM") as ps_big,
        ):
            # ---------- input DMAs, weights first so MM #1 can start ----
            wq_sb = cp.tile([128, 2, C], FP8, tag="wq", name="wq")
            wkv_sb = cp.tile([128, 2, 2 * C], FP8, tag="wkv", name="wkv")
            xc_sb, xw_sb = {}, {}
            for bb in range(BL):
                xc_sb[bb] = xcp.tile([128, NT, 2, 128], FP8, tag=f"xc{bb}",
                                     name=f"xc_{bb}")
                xw_sb[bb] = xwp.tile([128, 2, N], FP8, tag=f"xw{bb}",
                                     name=f"xw_{bb}")
            nc.sync.dma_start(out=xc_sb[0][:, 0:4], in_=xc[0, :, 0:4])
            nc.sync.dma_start(out=wkv_sb[:], in_=wkv[:])
            nc.sync.dma_start(out=wq_sb[:], in_=wq[:])
            nc.sync.dma_start(out=xc_sb[0][:, 4:NT], in_=xc[0, :, 4:NT])
            nc.sync.dma_start(out=xc_sb[1][:], in_=xc[1])
            nc.sync.dma_start(out=xw_sb[0][:], in_=xw[0])
            nc.sync.dma_start(out=xw_sb[1][:], in_=xw[1])

            # kt/vt staging tiles, shared across batches; ones column of
            # vt written once, never touched again
            kt8 = [ktp.tile([128, 2, C], FP8, tag=f"kt{p}", name=f"kt_{p}")
                   for p in range(NPAIR)]
            vt8 = [vtp.tile([128, 2, C + 1], FP8, tag=f"vt{p}", name=f"vt_{p}")
                   for p in range(NPAIR)]
            for p in range(NPAIR):
                for j in range(2):
                    nc.gpsimd.memset(vt8[p][:, j, C:C + 1], 1.0)

            # ---------- Z-phase pair emitter --------------------------
            zstate = {}

            def emit_z_pair(zb, q8z, m8z, p, glen=8):
                if p % (glen // 2) == 0:
                    zstate["sb"] = hp.tile([128, glen, C + 1], BF16, tag="z",
                                           name=f"z_{zb}_{p}")
                z_sb = zstate["sb"]
                jp = p % (glen // 2)
                zps = ps_big.tile([128, 2, 512], F32, tag="big",
                                  name=f"zps_{zb}_{p}")
                for j in range(2):
                    nc.tensor.matmul(zps[:, j, 0:C + 1],
                                     lhsT=q8z[:, 2 * p + j, :, :],
                                     rhs=m8z[:],
                                     start=True, stop=True, perf_mode=DR)
                if p % 2 == 0:
                    nc.scalar.activation(z_sb[:, 2 * jp:2 * jp + 2, :],
                                         zps[:, :, 0:C + 1], Copy)
                else:
                    nc.vector.tensor_copy(z_sb[:, 2 * jp:2 * jp + 2, :],
                                          zps[:, :, 0:C + 1])
                if jp == glen // 2 - 1:
                    g0 = p - jp
                    nc.sync.dma_start(out=zs[zb, :, 2 * g0:2 * g0 + glen, :],
                                      in_=z_sb[:])

            # ---------- Q-phase emitter (wide rhs, chunk-major out) ----
            def emit_q(b, q8c, qi):
                mc, iw2 = qi // 4, qi % 4
                qps = ps_big.tile([128, 2, 512], F32, tag="big",
                                  name=f"qps_{b}_{qi}")
                for j in range(2):
                    nc.tensor.matmul(
                        qps[:, j, :],
                        lhsT=wq_sb[:, :, mc * 128:(mc + 1) * 128],
                        rhs=xw_sb[b][:, :, (iw2 * 2 + j) * 512:(iw2 * 2 + j + 1) * 512],
                        start=True, stop=True, perf_mode=DR)
                # dst: chunks 8*iw2 + 4*j + h, c-half mc  (4D view of src)
                src = qps[:, :, :].rearrange("p j (h r) -> p j h r", r=128)
                dst = q8c[:, 8 * iw2:8 * iw2 + 8, mc, :].rearrange(
                    "p (j h) r -> p j h r", j=2)
                if qi % 2 == 0:
                    nc.scalar.activation(dst, src, Relu)
                else:
                    nc.vector.tensor_scalar_max(dst, src, 0.0)

            prev = None  # (q8c, m8) of the previous batch
            for b in range(BL):
                # ---------- window A: K(b) zipped with Z(b-1) ----------
                for p in range(NPAIR):
                    kvps = ps_big.tile([128, 2, 512], F32, tag="big",
                                       name=f"kvps_{b}_{p}")
                    for j in range(2):
                        nc.tensor.matmul(kvps[:, j, :],
                                         lhsT=xc_sb[b][:, 2 * p + j, :, :],
                                         rhs=wkv_sb[:], start=True, stop=True,
                                         perf_mode=DR)
                    if p % 2 == 0:
                        nc.scalar.activation(kt8[p][:, :, :],
                                             kvps[:, :, 0:C], Relu)
                        nc.vector.tensor_copy(vt8[p][:, :, 0:C],
                                              kvps[:, :, C:2 * C])
                    else:
                        nc.vector.tensor_scalar_max(kt8[p][:, :, :],
                                                    kvps[:, :, 0:C], 0.0)
                        nc.scalar.activation(vt8[p][:, :, 0:C],
                                             kvps[:, :, C:2 * C], Copy)
                    # Z(b-1) pairs 0..7 ride the K window (one per odd slot)
                    if prev is not None and p % 2 == 1:
                        emit_z_pair(b - 1, prev[0], prev[1], p // 2)

                # ---------- window B: kv(b) zipped with Q(b), Z 8..15 --
                q8c = qp.tile([128, NT, 2, 128], FP8, tag="q", name=f"q_{b}")
                kv_ps = ps_big.tile([128, 2, 512], F32, tag="big",
                                    name=f"kv_{b}")
                for p in range(NPAIR):
                    for cc in range(2):
                        nc.tensor.matmul(kv_ps[:, cc, 0:C + 1],
                                         lhsT=kt8[p][:, :, cc * 128:(cc + 1) * 128],
                                         rhs=vt8[p][:],
                                         start=(p == 0), stop=(p == NPAIR - 1),
                                         perf_mode=DR)
                    if p % 2 == 1:
                        emit_q(b, q8c, p // 2)
                    elif prev is not None:
                        emit_z_pair(b - 1, prev[0], prev[1], 8 + p // 2)
                m8 = sp.tile([128, 2, C + 1], FP8, tag="m8", name=f"m8_{b}")
                nc.scalar.activation(m8[:], kv_ps[:, :, 0:C + 1], Copy, scale=S)
                prev = (q8c, m8)

            # ---------- final batch's Z phase, tapered groups ----------
            for p in range(NPAIR):
                emit_z_pair(BL - 1, prev[0], prev[1], p, glen=4)
    nc.compile()
    return nc


def _prep_inputs(x, w_qkv, w_proj, bn_gamma, bn_beta, bn_mean, bn_var):
    x = np.asarray(x, dtype=np.float32)
    w_qkv = np.asarray(w_qkv, dtype=np.float32)
    w_proj = np.asarray(w_proj, dtype=np.float32)
    bn_gamma = np.asarray(bn_gamma, dtype=np.float32)
    bn_beta = np.asarray(bn_beta, dtype=np.float32)
    bn_mean = np.asarray(bn_mean, dtype=np.float32)
    bn_var = np.asarray(bn_var, dtype=np.float32)

    # torch-faithful interleave: out-channel 3*i+j -> (channel i, {q,k,v}[j])
    def w8(wm):  # (C_out, C_in) -> [128, 2, C_out] fp8: [p, j, o] = w[o, j*128+p]
        return np.ascontiguousarray(
            wm.T.reshape(2, 128, -1).transpose(1, 0, 2).astype(NPFP8))

    scale = bn_gamma / np.sqrt(bn_var + 1e-5)
    wq8 = w8(w_qkv[0::3])
    # combined [wk | Wp'.Wv] along the output dim (projection folded into V)
    wpv = (scale[:, None] * w_proj) @ w_qkv[2::3]
    wkv8 = w8(np.concatenate([w_qkv[1::3], wpv], axis=0))
    x8 = x.reshape(B, 2, 128, N).astype(NPFP8)
    # wide: [b, p, j, n] = x[b, j*128+p, n]
    xw8 = np.ascontiguousarray(x8.transpose(0, 2, 1, 3))
    # chunk-major: [b, p, i, j, r] = x[b, j*128+p, i*128+r]
    xc8 = np.ascontiguousarray(
        x8.reshape(B, 2, 128, NT, 128).transpose(0, 2, 3, 1, 4))

    bias = (bn_beta - bn_mean * scale).astype(np.float32)

    in_maps = []
    for core in range(NCORES):
        in_maps.append({
            "xc8": xc8[core * BL:(core + 1) * BL],
            "xw8": xw8[core * BL:(core + 1) * BL],
            "wq8": wq8, "wkv8": wkv8,
        })
    return in_maps, bias


def _postprocess(z_raw, bias):
    # z_raw: (B, 128, NT, C+1), n = i*128+p -> y (B, C, H, W) f32
    z = z_raw.transpose(0, 2, 1, 3).reshape(B, N, C + 1)
    y = z[:, :, :C] / z[:, :, C:C + 1] + bias[None, None, :]
    return np.ascontiguousarray(y.transpose(0, 2, 1)).reshape(B, C, H, W)


def _run(inputs, trace=False, **kw):
    if "nc" not in _CACHE:
        _CACHE["nc"] = _build_program()
    nc = _CACHE["nc"]
    in_maps, bias = _prep_inputs(**inputs)
    res = run_bass_kernel_spmd(nc, in_maps, list(range(NCORES)), trace=trace, **kw)
    z_raw = np.concatenate([res.results[i]["z"] for i in range(NCORES)], axis=0)
    return _postprocess(z_raw.astype(np.float32), bias), res


def kernel(**inputs):
    y, _ = _run(inputs)
    return y
